# revision 23
# baseline (speedup 1.0000x reference)
"""AttnBlock (GroupNorm + single-head self-attention + residual) on 8 trn2 cores.

Problem: X [4, 512, 64, 64] f32. Per batch element: GroupNorm(32 groups), then
1x1-conv Q/K/V projections, softmax attention over n=h*w=4096 positions,
proj_out, residual add.

Sharding: 8 cores = 4 batch elements x 2 query-halves. Each core computes the
full GroupNorm + K/V for its batch element (duplicated within the pair) and
attention output for its 2048-query half.

Layout strategy (per core):
  Hn, K, Q kept channel-major [c, n] (c on partitions)  -> projections are
  natural matmuls.  S^T[k, q] = sum_c K[c,k] Q[c,q] computed with k on
  partitions so softmax sums reduce via a ones-vector matmul on the PE and
  Ho[q, c] = sum_k expS[k,q] V[k,c] accumulates flash-style in PSUM without
  ever materializing/transposing the 4096x4096 attention matrix.
  Softmax skips max-subtraction: |S*scale| < ~10 here, exp is safe in f32.

All big matmuls run in float32r (full PE rate at N=512, ~1.5e-4 rel err).

SBUF (208KB/partition) forces a two-pass GroupNorm: pass 1 streams X for
stats only; pass 2 re-reads X in halves, normalizes, and immediately
projects K (staged to DRAM scratch) and V.  Q likewise from the Xq input.
K is reloaded into SBUF for the attention phase once Hn is gone.
"""

import numpy as np

B, C, H, W = 4, 512, 64, 64
N = H * W            # 4096 keys per batch element
NQ = N // 2          # 2048 queries per core
CT = C // 128        # 4 channel tiles
NT = N // 128        # 32 key tiles
QC = NQ // 512       # 4 query chunks of 512
GROUPS = 32
GPT = GROUPS // CT   # 8 groups per 128-channel tile
GSZ = C // GROUPS    # 16 channels per group
EPS = 1e-5
SCALE = float(C) ** -0.5

_CACHE = {}


def _build(debug=False):
    from contextlib import ExitStack
    from concourse import bacc
    import concourse.mybir as mybir
    import concourse.tile as tile
    from concourse.masks import make_identity

    f32 = mybir.dt.float32
    f32r = mybir.dt.float32r
    AF = mybir.ActivationFunctionType
    OP = mybir.AluOpType

    nc = bacc.Bacc()
    X = nc.dram_tensor("X", [C, N], f32, kind="ExternalInput")
    Xq = nc.dram_tensor("Xq", [C, NQ], f32, kind="ExternalInput")
    wT = {
        nm: nc.dram_tensor(nm, [C, C], f32, kind="ExternalInput")
        for nm in ("wqT", "wkT", "wvT", "wpT")
    }
    vecs = {
        nm: nc.dram_tensor(nm, [C], f32, kind="ExternalInput")
        for nm in ("bq", "bk", "bpe", "gn_w", "gn_b")
    }
    gmat_d = nc.dram_tensor("gmat_d", [128, GPT], f32, kind="ExternalInput")
    ones2_d = nc.dram_tensor("ones2_d", [128, 2], f32, kind="ExternalInput")
    gmatT_d = nc.dram_tensor("gmatT_d", [GPT, 128], f32, kind="ExternalInput")
    out = nc.dram_tensor("out", [C, NQ], f32, kind="ExternalOutput")
    dbg = {}
    if debug:
        for nm, shp in [("dbg_scbi", [128, 2 * CT]), ("dbg_q", [128, 512]),
                        ("dbg_k", [128, 512]), ("dbg_v", [128, C]),
                        ("dbg_es", [128, 512]), ("dbg_sums", [128, 8]),
                        ("dbg_ho", [128, 512]), ("dbg_hoT", [128, 512]),
                        ("dbg_sraw", [128, 512])]:
            dbg[nm] = nc.dram_tensor(nm, shp, f32, kind="ExternalOutput")

    def col(v, ci):
        # [C] dram vector -> [128, 1] AP for channel tile ci
        return vecs[v][ci * 128:(ci + 1) * 128].rearrange("(p one) -> p one", one=1)

    def load_f32r(pool, stage_pool, dram_ap, shape, tag):
        """DMA f32 -> staging, DVE-convert -> f32r tile (real format change)."""
        st = stage_pool.tile(shape, f32, tag="ld_stage", name="ld_stage")
        nc.sync.dma_start(out=st, in_=dram_ap)
        t = pool.tile(shape, f32r, tag=tag, name=tag)
        nc.vector.tensor_copy(out=t, in_=st)
        return t

    # fp32r is an opaque on-chip format: every fp32r operand must be produced
    # by a compute-engine conversion (DVE copy), never by a bitcast DMA.

    with tile.TileContext(nc) as tc, ExitStack() as ctx:
        consts = ctx.enter_context(tc.tile_pool(name="consts", bufs=1))
        dpool = ctx.enter_context(tc.tile_pool(name="dram", bufs=1, space="DRAM"))
        pp_acc = ctx.enter_context(tc.tile_pool(name="pp_acc", bufs=4, space="PSUM"))
        pp_sps = ctx.enter_context(tc.tile_pool(name="pp_sps", bufs=3, space="PSUM"))
        pp_sums = ctx.enter_context(tc.tile_pool(name="pp_sums", bufs=1, space="PSUM"))

        k_dram = dpool.tile([C, N], f32r, tag="k_dram", name="k_dram")

        # ---- constants ----
        ident = consts.tile([128, 128], f32, tag="ident", name="ident")
        make_identity(nc, ident)
        with tc.tile_pool(name="cstage", bufs=2) as cstage:
            gmat = load_f32r(consts, cstage, gmat_d[:, :], [128, GPT], "gmat")
            gmatT = load_f32r(consts, cstage, gmatT_d[:, :], [GPT, 128], "gmatT")
            ones_col = load_f32r(consts, cstage, ones2_d[:, :], [128, 2], "ones")
        eps_t = consts.tile([128, 1], f32, tag="eps", name="eps")
        nc.vector.memset(eps_t, EPS)
        vt = {}
        for nm in ("bq", "bk", "bpe", "gn_w", "gn_b"):
            vt[nm] = consts.tile([128, CT], f32, tag=nm, name=nm)
            for ci in range(CT):
                nc.sync.dma_start(out=vt[nm][:, ci:ci + 1], in_=col(nm, ci))
        # per-row GN affine: hn = x * sc_all[:,ci] + bi_all[:,ci]
        sc_all = consts.tile([128, CT], f32, tag="sc_all", name="sc_all")
        bi_all = consts.tile([128, CT], f32, tag="bi_all", name="bi_all")
        # proj weights stay resident (needed at the very end)
        wpT_sb = []
        with tc.tile_pool(name="wstage", bufs=2) as wstage:
            for ci in range(CT):
                wpT_sb.append(load_f32r(
                    consts, wstage, wT["wpT"][ci * 128:(ci + 1) * 128, :],
                    [128, C], f"wpT{ci}"))

        q_sb = [consts.tile([128, NQ], f32r, tag=f"q{co}", name=f"q{co}")
                for co in range(CT)]
        v_sb = [consts.tile([128, C], f32r, tag=f"v{nt}", name=f"v{nt}")
                for nt in range(NT)]

        def apply_gn(t, ci):
            nc.vector.tensor_scalar(out=t, in0=t.bitcast(f32),
                                    scalar1=sc_all[:, ci:ci + 1],
                                    scalar2=bi_all[:, ci:ci + 1],
                                    op0=OP.mult, op1=OP.add)

        # ---- GroupNorm + Q + K/V projections (X resident once) ----
        with tc.tile_pool(name="hnx", bufs=1) as hnx:
          with tc.tile_pool(name="gn_stats", bufs=2) as gstats:
            # stats for all 4 channel tiles, then ONE batched group-reduce +
            # affine chain (the per-ci chains are tiny latency-bound ops).
            hn = []
            rowst_all = gstats.tile([128, CT, 2], f32r, tag="rowst", name="rowst")
            with nc.named_scope("gn"):
                for ci in range(CT):
                    t = hnx.tile([128, N], f32r, tag=f"xh{ci}", name=f"xh{ci}")
                    # split the DMA so bn_stats starts at the first half
                    for nh in range(2):
                        nc.sync.dma_start(
                            out=t[:, nh * NQ:(nh + 1) * NQ],
                            in_=X[ci * 128:(ci + 1) * 128,
                                  nh * NQ:(nh + 1) * NQ].bitcast(f32r))
                    hn.append(t)
                    xt = t.bitcast(f32)
                    stats = gstats.tile([128, N // 512, 6], f32, tag="bnst",
                                        name="bnst")
                    for s in range(N // 512):
                        nc.vector.bn_stats(out=stats[:, s, :],
                                           in_=xt[:, s * 512:(s + 1) * 512])
                    mv = gstats.tile([128, 2], f32, tag="mv", name="mv")
                    nc.vector.bn_aggr(out=mv, in_=stats)
                    # rowstats = [mean, E[x^2]] ; E[x^2] = var + mean^2
                    nc.vector.tensor_copy(out=rowst_all[:, ci, 0:1],
                                          in_=mv[:, 0:1])
                    m2 = gstats.tile([128, 1], f32, tag="m2", name="m2")
                    nc.vector.tensor_mul(out=m2, in0=mv[:, 0:1], in1=mv[:, 0:1])
                    nc.vector.tensor_add(out=rowst_all[:, ci, 1:2],
                                         in0=mv[:, 1:2], in1=m2)

                # group-reduce 128 rows -> 8 groups -> broadcast, all ci at once
                gps = pp_sps.tile([GPT, CT, 2], f32, tag="s_ps", name="gps")
                nc.tensor.matmul(out=gps, lhsT=gmat,
                                 rhs=rowst_all.rearrange("p c two -> p (c two)"),
                                 start=True, stop=True)
                gsb = gstats.tile([GPT, CT * 2], f32r, tag="gsb", name="gsb")
                nc.vector.tensor_copy(out=gsb,
                                      in_=gps.rearrange("g c two -> g (c two)"))
                bps = pp_sps.tile([128, CT, 2], f32, tag="s_ps", name="bps")
                nc.tensor.matmul(out=bps, lhsT=gmatT, rhs=gsb,
                                 start=True, stop=True)
                gstat = gstats.tile([128, CT, 2], f32, tag="gstat", name="gstat")
                nc.scalar.mul(out=gstat, in_=bps, mul=1.0 / GSZ)

                means = gstat[:, :, 0:1].rearrange("p c one -> p (c one)")
                m2s = gstat[:, :, 1:2].rearrange("p c one -> p (c one)")
                var = gstats.tile([128, CT], f32, tag="var", name="var")
                mm_ = gstats.tile([128, CT], f32, tag="mm_", name="mm_")
                nc.vector.tensor_mul(out=mm_, in0=means, in1=means)
                nc.vector.tensor_sub(out=var, in0=m2s, in1=mm_)
                # rstd = 1/sqrt(var + eps)
                nc.scalar.activation(out=var, in_=var, func=AF.Sqrt,
                                     bias=eps_t, scale=1.0)
                rstd = gstats.tile([128, CT], f32, tag="rstd", name="rstd")
                nc.vector.reciprocal(out=rstd, in_=var)
                # sc = rstd * gn_w ; bi = gn_b - mean * sc
                nc.vector.tensor_mul(out=sc_all, in0=rstd, in1=vt["gn_w"])
                msc = gstats.tile([128, CT], f32, tag="msc", name="msc")
                nc.vector.tensor_mul(out=msc, in0=means, in1=sc_all)
                nc.vector.tensor_sub(out=bi_all, in0=vt["gn_b"], in1=msc)

                for ci in range(CT):
                    apply_gn(hn[ci], ci)

          # ---- Q first (fills PE while GN tail + K/V weights stream) ----
          with tc.tile_pool(name="wq", bufs=1) as wqp:
            wq_sb = []
            for ci in range(CT):
                wq_sb.append(load_f32r(
                    wqp, wqp, wT["wqT"][ci * 128:(ci + 1) * 128, :],
                    [128, C], f"wq{ci}"))
            for half in range(2):
                with tc.tile_pool(name="hq_half", bufs=1) as hqpool:
                    hq = []
                    for ci in range(CT):
                        t = hqpool.tile([128, NQ // 2], f32r, tag=f"xq{ci}",
                                        name=f"xq{ci}")
                        nc.sync.dma_start(
                            out=t,
                            in_=Xq[ci * 128:(ci + 1) * 128,
                                   half * (NQ // 2):(half + 1) * (NQ // 2)
                                   ].bitcast(f32r))
                        apply_gn(t, ci)
                        hq.append(t)
                    with nc.named_scope("qproj"):
                        for co in range(CT):
                            for qn in range(NQ // 1024):
                                ps = pp_acc.tile([128, 512], f32, tag="acc",
                                                 name="acc")
                                for ci in range(CT):
                                    nc.tensor.matmul(
                                        out=ps,
                                        lhsT=wq_sb[ci][:, co * 128:(co + 1) * 128],
                                        rhs=hq[ci][:, qn * 512:(qn + 1) * 512],
                                        start=(ci == 0), stop=(ci == CT - 1))
                                nc.vector.tensor_scalar_add(
                                    out=q_sb[co][:, half * (NQ // 2) + qn * 512:
                                                 half * (NQ // 2) + (qn + 1) * 512],
                                    in0=ps, scalar1=vt["bq"][:, co:co + 1])

          with tc.tile_pool(name="wkv", bufs=1) as wkv, \
               tc.tile_pool(name="stage", bufs=4) as stage:
            wk_sb, wv_sb = [], []
            for ci in range(CT):
                wk_sb.append(load_f32r(
                    wkv, stage, wT["wkT"][ci * 128:(ci + 1) * 128, :],
                    [128, C], f"wk{ci}"))
                wv_sb.append(load_f32r(
                    wkv, stage, wT["wvT"][ci * 128:(ci + 1) * 128, :],
                    [128, C], f"wv{ci}"))
            with nc.named_scope("kproj"):
                for co in range(CT):
                    for kn in range(N // 512):
                        ps = pp_acc.tile([128, 512], f32, tag="acc", name="acc")
                        for ci in range(CT):
                            nc.tensor.matmul(
                                out=ps, lhsT=wk_sb[ci][:, co * 128:(co + 1) * 128],
                                rhs=hn[ci][:, kn * 512:(kn + 1) * 512],
                                start=(ci == 0), stop=(ci == CT - 1))
                        st = stage.tile([128, 512], f32r, tag="kst", name="kst")
                        nc.vector.tensor_scalar_add(out=st, in0=ps,
                                                    scalar1=vt["bk"][:, co:co + 1])
                        nc.sync.dma_start(
                            out=k_dram[co * 128:(co + 1) * 128,
                                       kn * 512:(kn + 1) * 512],
                            in_=st)
            with nc.named_scope("vproj"):
                for nt in range(NT):
                    ps = pp_acc.tile([128, 512], f32, tag="acc", name="acc")
                    for ci in range(CT):
                        nc.tensor.matmul(
                            out=ps, lhsT=hn[ci][:, nt * 128:(nt + 1) * 128],
                            rhs=wv_sb[ci],
                            start=(ci == 0), stop=(ci == CT - 1))
                    nc.vector.tensor_copy(out=v_sb[nt], in_=ps)

        if debug:
            dt_ = consts.tile([128, 2 * CT], f32, tag="dbg1", name="dbg1")
            nc.vector.tensor_copy(out=dt_[:, :CT], in_=sc_all)
            nc.vector.tensor_copy(out=dt_[:, CT:], in_=bi_all)
            nc.sync.dma_start(out=dbg["dbg_scbi"][:, :], in_=dt_)
            dq = consts.tile([128, 512], f32, tag="dbg_q", name="dbg_q")
            nc.vector.tensor_copy(out=dq, in_=q_sb[0][:, :512])
            nc.sync.dma_start(out=dbg["dbg_q"][:, :], in_=dq)
            dv = consts.tile([128, C], f32, tag="dbg_v", name="dbg_v")
            nc.vector.tensor_copy(out=dv, in_=v_sb[0])
            nc.sync.dma_start(out=dbg["dbg_v"][:, :], in_=dv)

        # ---- attention ----
        with tc.tile_pool(name="kpool", bufs=1) as kpool, \
             tc.tile_pool(name="work", bufs=2) as work:
            k_sb = []
            for ci in range(CT):
                t = kpool.tile([128, N], f32r, tag=f"k{ci}", name=f"k{ci}")
                nc.sync.dma_start(out=t, in_=k_dram[ci * 128:(ci + 1) * 128, :])
                k_sb.append(t)
            if debug:
                dk = work.tile([128, 512], f32, tag="dbg_k", name="dbg_k", bufs=1)
                nc.vector.tensor_copy(out=dk, in_=k_sb[0][:, :512])
                nc.sync.dma_start(out=dbg["dbg_k"][:, :], in_=dk)

            for qc in range(QC):
                qs = slice(qc * 512, (qc + 1) * 512)
                ho_ps = [pp_acc.tile([128, 512], f32, tag="acc", name="acc")
                         for _ in range(4)]
                sums_ps = pp_sums.tile([128, 8], f32, tag="sums", name="sums")
                nc.vector.memset(sums_ps, 0.0)
                for kt in range(NT):
                    s_ps = pp_sps.tile([128, 512], f32, tag="s_ps", name="s_ps")
                    with nc.named_scope("attn_s"):
                        for ci in range(CT):
                            nc.tensor.matmul(
                                out=s_ps, lhsT=k_sb[ci][:, kt * 128:(kt + 1) * 128],
                                rhs=q_sb[ci][:, qs],
                                start=(ci == 0), stop=(ci == CT - 1))
                    es = work.tile([128, 512], f32r, tag="es", name="es", bufs=4 if debug else 6)
                    if debug and qc == 0 and kt == 0:
                        dsr = work.tile([128, 512], f32, tag="dbg_sraw", name="dbg_sraw", bufs=1)
                        nc.vector.tensor_copy(out=dsr, in_=s_ps)
                        nc.sync.dma_start(out=dbg["dbg_sraw"][:, :], in_=dsr)
                    nc.scalar.activation(out=es, in_=s_ps, func=AF.Exp, scale=SCALE)
                    if debug and qc == 0 and kt == 0:
                        des = work.tile([128, 512], f32, tag="dbg_es", name="dbg_es", bufs=1)
                        nc.vector.tensor_copy(out=des, in_=es)
                        nc.sync.dma_start(out=dbg["dbg_es"][:, :], in_=des)
                    with nc.named_scope("attn_ho"):
                        for j in range(4):
                            nc.tensor.matmul(
                                out=ho_ps[j], lhsT=es[:, j * 128:(j + 1) * 128],
                                rhs=v_sb[kt],
                                start=(kt == 0), stop=(kt == NT - 1))
                            nc.tensor.matmul(
                                out=sums_ps[:, 2 * j:2 * j + 2],
                                lhsT=es[:, j * 128:(j + 1) * 128], rhs=ones_col,
                                start=False, stop=(kt == NT - 1),
                                skip_group_check=True)

                inv = work.tile([128, 8], f32, tag="inv", name="inv")
                nc.vector.reciprocal(out=inv, in_=sums_ps)
                if debug and qc == 0:
                    nc.sync.dma_start(out=dbg["dbg_sums"][:, :], in_=inv)

                hoT = [work.tile([128, 512], f32r, tag="hoT", name="hoT", bufs=4 if debug else 5)
                       for _ in range(CT)]
                scope_tail = nc.enter_named_scope("attn_tail", False)
                for j in range(4):
                    ho_sb = work.tile([128, 512], f32, tag="ho_sb", name="ho_sb", bufs=1 if debug else 2)
                    nc.vector.tensor_scalar_mul(out=ho_sb, in0=ho_ps[j],
                                                scalar1=inv[:, 2 * j:2 * j + 1])
                    if debug and qc == 0 and j == 0:
                        nc.sync.dma_start(out=dbg["dbg_ho"][:, :], in_=ho_sb)
                    for ci in range(CT):
                        tp = pp_sps.tile([128, 128], f32, tag="s_ps", name="tp")
                        nc.tensor.transpose(tp, ho_sb[:, ci * 128:(ci + 1) * 128],
                                            ident)
                        nc.vector.tensor_copy(
                            out=hoT[ci][:, j * 128:(j + 1) * 128], in_=tp)

                if debug and qc == 0:
                    dht = work.tile([128, 512], f32, tag="dbg_hoT", name="dbg_hoT", bufs=1)
                    nc.vector.tensor_copy(out=dht, in_=hoT[0])
                    nc.sync.dma_start(out=dbg["dbg_hoT"][:, :], in_=dht)
                nc.leave_named_scope("attn_tail", scope_tail[0], False)
                for co in range(CT):
                    ps = pp_acc.tile([128, 512], f32, tag="acc", name="acc")
                    for ci in range(CT):
                        nc.tensor.matmul(
                            out=ps, lhsT=wpT_sb[ci][:, co * 128:(co + 1) * 128],
                            rhs=hoT[ci],
                            start=(ci == 0), stop=(ci == CT - 1))
                    xr = work.tile([128, 512], f32, tag="xr", name="xr", bufs=1 if debug else 2)
                    nc.sync.dma_start(out=xr, in_=Xq[co * 128:(co + 1) * 128, qs])
                    ot = work.tile([128, 512], f32, tag="ot", name="ot", bufs=1 if debug else 2)
                    nc.vector.tensor_scalar_add(out=ot, in0=ps,
                                                scalar1=vt["bpe"][:, co:co + 1])
                    nc.vector.tensor_add(out=ot, in0=ot, in1=xr)
                    nc.sync.dma_start(out=out[co * 128:(co + 1) * 128, qs], in_=ot)

    nc.compile()
    return nc


def _get_nc():
    if "nc" not in _CACHE:
        _CACHE["nc"] = _build()
    return _CACHE["nc"]


def _prep_in_maps(X, gn_w, gn_b, wq, bq, wk, bk, wv, bv, wp, bp):
    X = np.ascontiguousarray(np.asarray(X, dtype=np.float32))
    f = lambda a: np.ascontiguousarray(np.asarray(a, dtype=np.float32))
    gn_w, gn_b, bq, bk, bv, bp = map(f, (gn_w, gn_b, bq, bk, bv, bp))
    wq, wk, wv, wp = map(f, (wq, wk, wv, wp))

    Xf = X.reshape(B, C, N)
    bpe = wp @ bv + bp  # bv folded through proj_out (sum_k softmax == 1)
    wqT = np.ascontiguousarray(wq.T)
    wkT = np.ascontiguousarray(wk.T)
    wvT = np.ascontiguousarray(wv.T)
    wpT = np.ascontiguousarray(wp.T)

    gmat = np.zeros((128, GPT), np.float32)
    for g in range(GPT):
        gmat[g * GSZ:(g + 1) * GSZ, g] = 1.0
    gmatT = np.ascontiguousarray(gmat.T)

    in_maps = []
    for core in range(8):
        bi, half = core // 2, core % 2
        q0 = half * NQ
        Xb = Xf[bi]
        in_maps.append({
            "X": Xb,
            "Xq": np.ascontiguousarray(Xb[:, q0:q0 + NQ]),
            "wqT": wqT, "wkT": wkT, "wvT": wvT, "wpT": wpT,
            "bq": bq, "bk": bk, "bpe": bpe, "gn_w": gn_w, "gn_b": gn_b,
            "gmat_d": gmat, "gmatT_d": gmatT,
            "ones2_d": np.ones((128, 2), np.float32),
        })
    return in_maps


_last_in_maps = None


def kernel(X, gn_w, gn_b, wq, bq, wk, bk, wv, bv, wp, bp):
    from concourse.bass_utils import run_bass_kernel_spmd

    global _last_in_maps
    in_maps = _prep_in_maps(X, gn_w, gn_b, wq, bq, wk, bk, wv, bv, wp, bp)
    _last_in_maps = in_maps
    nc = _get_nc()
    res = run_bass_kernel_spmd(nc, in_maps, list(range(8)))
    out = np.empty((B, C, N), np.float32)
    for core in range(8):
        bi, half = core // 2, core % 2
        out[bi][:, half * NQ:(half + 1) * NQ] = res.results[core]["out"]
    return out.reshape(B, C, H, W)
